# revision 1
# baseline (speedup 1.0000x reference)
"""GQA attention (16 Q heads / 4 KV heads, RoPE, n=2048, d=64) on 8 trn2 cores.

Sharding: core c = (batch b=c//4, kv-group j=c%4). Each core owns 4 query
heads sharing one KV head, computes its partial output projection
(O_heads @ Wo_rows), and the host sums the 4 partials per batch.

All on-device layouts keep head_dim (or inner dim) on SBUF partitions so no
activation transposes are needed:
  qT [64, 4*2048]  (4 heads concatenated along free)
  kT [64, 2048]
  S^T [keys, queries] tiles from matmul(lhsT=kT_blk, rhs=qT_chunk)
  P^T = exp(S^T/8) on ACT
  O^T+denom from matmul(lhsT=V_aug[keys,65], rhs=P^T)  (ones col -> denom)
Matmul inputs are bf16 (1 cycle/row), accumulation fp32 in PSUM.
"""

import os
import sys
import functools

import numpy as np

sys.path.insert(0, "/opt/trn_rl_repo")

import concourse.bass as bass  # noqa: E402
import concourse.bacc as bacc  # noqa: E402
import concourse.tile as tile  # noqa: E402
import concourse.mybir as mybir  # noqa: E402
from concourse.masks import make_identity  # noqa: E402

F32 = mybir.dt.float32
BF16 = mybir.dt.bfloat16
EXP = mybir.ActivationFunctionType.Exp

B, N, DIM = 2, 2048, 1024
HEADS, KVH, D = 16, 4, 64
HPC = HEADS // KVH          # q heads per core = 4
SCALE = D ** -0.5           # 1/8
QTOT = HPC * N              # 8192 concatenated query columns
NKB = N // 128              # 16 key blocks
NDB = DIM // 128            # 8 contraction blocks for projections

LAST_RESULTS = {}           # test.py introspection


def build_kernel(nc, tc, io):
    from contextlib import ExitStack

    xt, wq, wkv, wo = io["xt"], io["wq"], io["wkv"], io["wo"]
    cost, sincat, out = io["cost"], io["sincat"], io["out"]

    es = ExitStack()
    consts = es.enter_context(tc.tile_pool(name="consts", bufs=1))
    ot_pool = es.enter_context(tc.tile_pool(name="ot", bufs=1))
    qk_pool = es.enter_context(tc.tile_pool(name="qk", bufs=1))

    # --- constants / weights in SBUF ---
    wq_sb = consts.tile([128, NDB, 2 * 128], BF16, tag="wq")      # 8KB/part
    wkv_sb = consts.tile([128, NDB, 128], BF16, tag="wkv")        # 4KB/part
    wo_sb = consts.tile([128, 2, DIM], BF16, tag="wo")            # 8KB/part
    cos_sb = consts.tile([64, N], F32, tag="cos")                # 8KB/part
    sin_sb = consts.tile([64, N], F32, tag="sin")                # 8KB/part
    id64 = consts.tile([64, 64], BF16, tag="id")
    nc.sync.dma_start(wq_sb, wq.transpose([1, 0, 2]))
    nc.sync.dma_start(wkv_sb, wkv.transpose([1, 0, 2]))
    nc.sync.dma_start(wo_sb, wo.transpose([1, 0, 2]))
    nc.sync.dma_start(cos_sb, cost)
    nc.sync.dma_start(sin_sb, sincat)
    make_identity(nc, id64)

    # --- activations ---
    qt_sb = qk_pool.tile([128, QTOT], BF16, tag="qt")             # 16KB/part
    kt_sb = qk_pool.tile([128, N], BF16, tag="kt")                # 4KB/part
    vaug_sb = qk_pool.tile([128, NKB, 128], BF16, tag="vaug")     # 4KB/part
    # zero the pad regions once: K rows 64:128 of qt/kt, V cols 65:128
    nc.gpsimd.memset(qt_sb[64:128, :], 0.0)
    nc.gpsimd.memset(kt_sb[64:128, :], 0.0)
    nc.gpsimd.memset(vaug_sb, 0.0)
    ot_sb = [
        ot_pool.tile([128, N], BF16, tag=f"ot{i}", name=f"ot{i}") for i in range(2)
    ]
    # vT staging aliases into ot_sb[0] (free until attention writes it)
    vt_sb = ot_sb[0][0:64, :]

    def rope(dst, src, ch, tmp_pool):
        """dst[64,512] (SBUF) <- RoPE(src[64,512] (PSUM)), position chunk ch."""
        cs = cos_sb[:, ch * 512:(ch + 1) * 512]
        sn = sincat_slice = sin_sb[:, ch * 512:(ch + 1) * 512]
        t1 = tmp_pool.tile([64, 512], F32, tag="t1")
        t2 = tmp_pool.tile([64, 512], F32, tag="t2")
        nc.vector.tensor_mul(t1, src, cs)
        nc.vector.tensor_mul(t2[0:32, :], src[32:64, :], sn[0:32, :])
        nc.vector.tensor_mul(t2[32:64, :], src[0:32, :], sn[32:64, :])
        nc.vector.tensor_add(dst, t1, t2)

    with (
        tc.tile_pool(name="xt", bufs=1) as xt_pool,
        tc.tile_pool(name="ropetmp", bufs=2) as rope_tmp,
        tc.tile_pool(name="pproj", bufs=3, space="PSUM") as pp,
    ):
        xt_sb = xt_pool.tile([128, NDB, N], BF16, tag="xt")       # 64KB/part
        for kb in range(NDB):
            for ch in range(4):
                nc.sync.dma_start(
                    xt_sb[:, kb, ch * 512:(ch + 1) * 512],
                    xt[kb, :, ch * 512:(ch + 1) * 512],
                )

        # KV projection (k rows 0:64, v rows 64:128 of the pack).
        for ch in range(4):
            pkv = pp.tile([128, 512], F32, tag="pj")
            for kb in range(NDB):
                nc.tensor.matmul(
                    pkv,
                    wkv_sb[:, kb, :],
                    xt_sb[:, kb, ch * 512:(ch + 1) * 512],
                    start=(kb == 0),
                    stop=(kb == NDB - 1),
                )
            rope(kt_sb[0:64, ch * 512:(ch + 1) * 512], pkv[0:64, :], ch, rope_tmp)
            nc.vector.tensor_copy(
                vt_sb[:, ch * 512:(ch + 1) * 512], pkv[64:128, :]
            )

        # V_aug: transpose vT -> [keys,64] blocks, append ones column.
        for t in range(NKB):
            ptr = pp.tile([128, 64], BF16, tag="pjt")
            nc.tensor.transpose(
                ptr[:, 0:64], vt_sb[:, t * 128:(t + 1) * 128], id64
            )
            nc.vector.tensor_copy(vaug_sb[:, t, 0:64], ptr[:, 0:64])
            nc.vector.memset(vaug_sb[:, t, 64:65], 1.0)

        # Q projection: 2 head-pairs, 4 n-chunks each.
        for pack in range(2):
            for ch in range(4):
                pq = pp.tile([128, 512], F32, tag="pj")
                for kb in range(NDB):
                    nc.tensor.matmul(
                        pq,
                        wq_sb[:, kb, pack * 128:(pack + 1) * 128],
                        xt_sb[:, kb, ch * 512:(ch + 1) * 512],
                        start=(kb == 0),
                        stop=(kb == NDB - 1),
                    )
                for hh in range(2):
                    h = pack * 2 + hh
                    rope(
                        qt_sb[0:64, h * N + ch * 512: h * N + (ch + 1) * 512],
                        pq[hh * 64:(hh + 1) * 64, :],
                        ch,
                        rope_tmp,
                    )

    # --- attention ---
    with (
        tc.tile_pool(name="small", bufs=2) as small,
        tc.tile_pool(name="ppool", bufs=3) as ppool,
        tc.tile_pool(name="psS", bufs=2, space="PSUM") as psS,
        tc.tile_pool(name="psO", bufs=2, space="PSUM") as psO,
    ):
        for qc in range(QTOT // 1024):  # 8 chunks of 1024 queries
            po_t = psO.tile([128, 1024], F32, tag="o")
            for kb in range(NKB):
                ps_t = psS.tile([128, 1024], F32, tag="s")
                for half in range(2):
                    nc.tensor.matmul(
                        ps_t[:, half * 512:(half + 1) * 512],
                        kt_sb[:, kb * 128:(kb + 1) * 128],
                        qt_sb[:, qc * 1024 + half * 512: qc * 1024 + (half + 1) * 512],
                        start=True,
                        stop=True,
                    )
                p_t = ppool.tile([128, 1024], BF16, tag="p")
                nc.scalar.activation(p_t, ps_t, EXP, bias=0.0, scale=SCALE)
                for half in range(2):
                    nc.tensor.matmul(
                        po_t[:, half * 512:(half + 1) * 512],
                        vaug_sb[:, kb, :],
                        p_t[:, half * 512:(half + 1) * 512],
                        start=(kb == 0),
                        stop=(kb == NKB - 1),
                        skip_group_check=True,
                    )
            # normalize: O^T / denom (denom on psum partition 64)
            h = qc // 2
            pair, row0 = h // 2, 64 * (h % 2)
            col0 = (qc % 2) * 1024
            rc = small.tile([1, 1024], F32, tag="rc")
            nc.vector.reciprocal(rc, po_t[64:65, :])
            bc = small.tile([64, 1024], F32, tag="bc")
            nc.gpsimd.partition_broadcast(bc, rc)
            nc.vector.tensor_mul(
                ot_sb[pair][row0:row0 + 64, col0:col0 + 1024],
                po_t[0:64, :],
                bc,
            )

    # --- output projection: out[q, :] = sum_pair O^T_pair.T @ Wo_pair ---
    with (
        tc.tile_pool(name="pout", bufs=3, space="PSUM") as pout,
        tc.tile_pool(name="ostage", bufs=3) as ostage,
    ):
        for qb in range(N // 128):
            for nch in range(2):
                pt = pout.tile([128, 512], F32, tag="po")
                for pair in range(2):
                    nc.tensor.matmul(
                        pt,
                        ot_sb[pair][:, qb * 128:(qb + 1) * 128],
                        wo_sb[:, pair, nch * 512:(nch + 1) * 512],
                        start=(pair == 0),
                        stop=(pair == 1),
                    )
                st = ostage.tile([128, 512], F32, tag="st")
                nc.vector.tensor_copy(st, pt)
                nc.sync.dma_start(
                    out[qb * 128:(qb + 1) * 128, nch * 512:(nch + 1) * 512], st
                )

    es.close()


def _rope_tables():
    inv_freq = 1.0 / (10000.0 ** (np.arange(0, D, 2, dtype=np.float64) / D))
    freqs = np.outer(np.arange(N, dtype=np.float64), inv_freq)  # [N, 32]
    cos_h = np.cos(freqs).astype(np.float32).T                  # [32, N]
    sin_h = np.sin(freqs).astype(np.float32).T                  # [32, N]
    cost = np.concatenate([cos_h, cos_h], 0)                    # [64, N]
    sincat = np.concatenate([-sin_h, sin_h], 0)                 # [64, N]
    return np.ascontiguousarray(cost), np.ascontiguousarray(sincat)


@functools.lru_cache(maxsize=1)
def _program():
    nc = bacc.Bacc(
        "TRN2", target_bir_lowering=False, debug=False, enable_asserts=False
    )
    io = {
        "xt": nc.dram_tensor("xt", [NDB, 128, N], BF16, kind="ExternalInput").ap(),
        "wq": nc.dram_tensor("wq", [NDB, 128, 256], BF16, kind="ExternalInput").ap(),
        "wkv": nc.dram_tensor("wkv", [NDB, 128, 128], BF16, kind="ExternalInput").ap(),
        "wo": nc.dram_tensor("wo", [2, 128, DIM], BF16, kind="ExternalInput").ap(),
        "cost": nc.dram_tensor("cost", [64, N], F32, kind="ExternalInput").ap(),
        "sincat": nc.dram_tensor("sincat", [64, N], F32, kind="ExternalInput").ap(),
        "out": nc.dram_tensor("out", [N, DIM], F32, kind="ExternalOutput").ap(),
    }
    with tile.TileContext(nc) as tc:
        build_kernel(nc, tc, io)
    nc.compile()
    return nc


def make_in_maps(x, Wq, Wkv, Wo):
    import ml_dtypes

    bf16 = ml_dtypes.bfloat16
    cost, sincat = _rope_tables()
    in_maps = []
    for c in range(8):
        b, j = c // 4, c % 4
        xt = np.ascontiguousarray(x[b].T).reshape(NDB, 128, N)
        wq_c = np.ascontiguousarray(Wq[:, 256 * j:256 * (j + 1)]).reshape(
            NDB, 128, 256
        )
        wkv_c = np.ascontiguousarray(
            np.concatenate(
                [Wkv[:, 64 * j:64 * (j + 1)],
                 Wkv[:, 256 + 64 * j:256 + 64 * (j + 1)]],
                axis=1,
            )
        ).reshape(NDB, 128, 128)
        wo_c = np.ascontiguousarray(Wo[256 * j:256 * (j + 1), :]).reshape(
            2, 128, DIM
        )
        in_maps.append(
            {
                "xt": xt.astype(bf16),
                "wq": wq_c.astype(bf16),
                "wkv": wkv_c.astype(bf16),
                "wo": wo_c.astype(bf16),
                "cost": cost,
                "sincat": sincat,
            }
        )
    return in_maps


def _install_ntff_hook():
    """Register the axon NTFF profiling hook that this image's antenv lacks."""
    import types

    if "antenv.axon_hooks" in sys.modules:
        return
    try:
        sys.path.append("/root/.axon_site")
        from trn_agent_boot.trn_boot import _ntff_profile_via_ctypes

        hook = _ntff_profile_via_ctypes("/opt/axon/libaxon_pjrt.so")
    except Exception:
        hook = None
    finally:
        try:
            sys.path.remove("/root/.axon_site")
        except ValueError:
            pass
    mod = types.ModuleType("antenv.axon_hooks")
    mod.get_axon_ntff_profile_hook = lambda: hook
    mod.set_axon_ntff_profile_hook = lambda h: None
    sys.modules["antenv.axon_hooks"] = mod
    # artifact upload needs bucket credentials this container lacks
    import concourse.bass_utils as bu

    bu.upload_artifacts = lambda tmpdir: "local://" + str(tmpdir)


def kernel(x, Wq, Wkv, Wo, bo):
    from concourse.bass_utils import run_bass_kernel_spmd

    _install_ntff_hook()
    nc = _program()
    in_maps = make_in_maps(x, Wq, Wkv, Wo)
    trace = bool(os.environ.get("KERNEL_TRACE"))
    res = run_bass_kernel_spmd(
        nc, in_maps, list(range(8)), trace=trace
    )
    LAST_RESULTS["res"] = res
    full = np.zeros((B, N, DIM), np.float32)
    for c in range(8):
        full[c // 4] += res.results[c]["out"]
    full += bo.astype(np.float32)
    return full



# revision 10
# speedup vs baseline: 1.0196x; 1.0196x over previous
"""GQA attention (16 Q heads / 4 KV heads, RoPE, n=2048, d=64) on 8 trn2 cores.

Sharding: core c = (batch b=c//4, kv-group j=c%4). Each core owns 4 query
heads sharing one KV head, computes its partial output projection
(O_heads @ Wo_rows), and the host sums the 4 partials per batch.

Pipeline per core (all matmul inputs bf16/fp8, fp32 PSUM accumulation):
  x DMA'd ch-major so the KV projection starts ~7us in.
  RoPE on DVE in bf16 2x mode (PSUM staged to SBUF via ACT copies).
  S^T = K^T Q as fp8-e4m3 DoubleRow matmuls (d=64 folded to 32 partitions
    x 2 k-slots) -- half the PE time of bf16; P = exp on ACT; O^T+denom
    from bf16 matmul with V_aug (ones column -> denominator).
  Output projection interleaved per 512-query chunk, fp16 output.
"""

import os
import sys
import functools

import numpy as np

sys.path.insert(0, "/opt/trn_rl_repo")

import concourse.bass as bass  # noqa: E402
import concourse.bacc as bacc  # noqa: E402
import concourse.tile as tile  # noqa: E402
import concourse.mybir as mybir  # noqa: E402
from concourse.masks import make_identity  # noqa: E402

F32 = mybir.dt.float32
BF16 = mybir.dt.bfloat16
F16 = mybir.dt.float16
FP8 = mybir.dt.float8e4
EXP = mybir.ActivationFunctionType.Exp
DROW = mybir.MatmulPerfMode.DoubleRow

B, N, DIM = 2, 2048, 1024
HEADS, KVH, D = 16, 4, 64
HPC = HEADS // KVH          # q heads per core = 4
SCALE = D ** -0.5           # 1/8
QTOT = HPC * N              # 8192 concatenated query columns
NKB = N // 128              # 16 key blocks
NDB = DIM // 128            # 8 contraction blocks for projections

USE_FP8 = os.environ.get("KERNEL_FP8") == "1"

LAST_RESULTS = {}           # test.py introspection


def _build(nc, tc, io):
    from contextlib import ExitStack

    xt, wq, wkv, wo = io["xt"], io["wq"], io["wkv"], io["wo"]
    cost, sincat, out = io["cost"], io["sincat"], io["out"]

    es = ExitStack()
    consts = es.enter_context(tc.tile_pool(name="consts", bufs=1))
    acts = es.enter_context(tc.tile_pool(name="acts", bufs=1))

    wq_sb = consts.tile([128, NDB, 256], BF16, tag="wq")
    wkv_sb = consts.tile([128, NDB, 128], BF16, tag="wkv")
    wo_sb = consts.tile([128, 2, DIM], BF16, tag="wo")
    cos2 = consts.tile([128, N], BF16, tag="cos2")
    sin2 = consts.tile([128, N], BF16, tag="sin2")
    id64 = consts.tile([64, 64], BF16, tag="id")

    vt_sb = acts.tile([64, N], BF16, tag="vt")
    vaug_sb = acts.tile([128, NKB, 65], BF16, tag="vaug")
    ot_sb = [
        acts.tile([128, N], BF16, tag=f"ot{i}", name=f"ot{i}") for i in range(2)
    ]
    if USE_FP8:
        k8 = acts.tile([32, 2, N], FP8, tag="k8")
        q8 = acts.tile([32, 2, QTOT], FP8, tag="q8")
    else:
        kt_sb = acts.tile([128, N], BF16, tag="kt")
        qt_sb = acts.tile([128, QTOT], BF16, tag="qt")

    with tc.tile_pool(name="xtp", bufs=1) as xt_pool:
        xt_sb = xt_pool.tile([128, NDB, N], BF16, tag="xt")

        # DMA issue order: KV weights + ch0 tables + ch0 x first.
        for g in range(4):
            nc.sync.dma_start(
                wkv_sb[:, 2 * g:2 * g + 2, :],
                wkv[2 * g:2 * g + 2].transpose([1, 0, 2]),
            )
        nc.sync.dma_start(cos2[0:64, 0:1024], cost[:, 0:1024])
        nc.sync.dma_start(sin2[0:64, 0:1024], sincat[:, 0:1024])
        for kb in range(NDB):
            nc.sync.dma_start(xt_sb[:, kb, 0:512], xt[kb, :, 0:512])
        for kb in range(NDB):
            nc.sync.dma_start(wq_sb[:, kb, :], wq[kb])
        nc.sync.dma_start(cos2[0:64, 1024:N], cost[:, 1024:N])
        nc.sync.dma_start(sin2[0:64, 1024:N], sincat[:, 1024:N])
        nc.sync.dma_start(cos2[64:128, 0:1024], cost[:, 0:1024])
        nc.sync.dma_start(sin2[64:128, 0:1024], sincat[:, 0:1024])
        nc.sync.dma_start(cos2[64:128, 1024:N], cost[:, 1024:N])
        nc.sync.dma_start(sin2[64:128, 1024:N], sincat[:, 1024:N])
        for ch in range(1, 4):
            for kb in range(NDB):
                nc.sync.dma_start(
                    xt_sb[:, kb, ch * 512:(ch + 1) * 512],
                    xt[kb, :, ch * 512:(ch + 1) * 512],
                )
        for pair in range(2):
            for half in range(2):
                nc.sync.dma_start(
                    wo_sb[:, pair, half * 512:(half + 1) * 512],
                    wo[pair][:, half * 512:(half + 1) * 512],
                )
        make_identity(nc, id64)
        if not USE_FP8:
            nc.gpsimd.memset(kt_sb[64:128, :], 0.0)
            nc.gpsimd.memset(qt_sb[64:128, :], 0.0)

        with (
            tc.tile_pool(name="stage", bufs=3) as stage,
            tc.tile_pool(name="rtmp", bufs=3) as rtmp,
            tc.tile_pool(name="pq", bufs=1, space="PSUM") as pqp,
        ):
            with (
                tc.tile_pool(name="pkv", bufs=2, space="PSUM") as pp,
                tc.tile_pool(name="ptr", bufs=2, space="PSUM") as ptp,
            ):
                # ---- KV projection + K rope + V staging ----
                for ch in range(4):
                    cs = slice(ch * 512, (ch + 1) * 512)
                    pkv = pp.tile([128, 512], F32, tag="pj")
                    for kb in range(NDB):
                        nc.tensor.matmul(
                            pkv,
                            wkv_sb[:, kb, :],
                            xt_sb[:, kb, cs],
                            start=(kb == 0),
                            stop=(kb == NDB - 1),
                        )
                    ks = stage.tile([64, 512], BF16, tag="ks")
                    nc.vector.tensor_copy(ks, pkv[0:64, :])
                    nc.vector.tensor_copy(vt_sb[:, cs], pkv[64:128, :])
                    t1 = rtmp.tile([64, 512], BF16, tag="t1")
                    t2 = rtmp.tile([64, 512], BF16, tag="t2")
                    nc.vector.tensor_mul(t1, ks, cos2[0:64, cs])
                    nc.vector.tensor_mul(t2[0:32, :], ks[32:64, :], sin2[32:64, cs])
                    nc.vector.tensor_mul(t2[32:64, :], ks[0:32, :], sin2[0:32, cs])
                    if USE_FP8:
                        nc.vector.tensor_add(k8[:, 0, cs], t1[0:32, :], t2[0:32, :])
                        nc.vector.tensor_add(k8[:, 1, cs], t1[32:64, :], t2[32:64, :])
                    else:
                        nc.vector.tensor_add(kt_sb[0:64, cs], t1, t2)

                # ---- V_aug: transpose vT -> [keys,64] blocks + ones column ----
                for t in range(NKB):
                    ptr = ptp.tile([128, 64], BF16, tag="pjt")
                    nc.tensor.transpose(
                        ptr, vt_sb[:, t * 128:(t + 1) * 128], id64
                    )
                    nc.vector.tensor_copy(vaug_sb[:, t, 0:64], ptr)
                    nc.vector.memset(vaug_sb[:, t, 64:65], 1.0)

            # ---- Q projection + rope (pipelined into the attention rounds) ----
            def q_proj_ch(ch):
                cs = slice(ch * 512, (ch + 1) * 512)
                for pack in range(2):
                    pq = pqp.tile([128, 512], F32, tag="pj")
                    for kb in range(NDB):
                        nc.tensor.matmul(
                            pq,
                            wq_sb[:, kb, pack * 128:(pack + 1) * 128],
                            xt_sb[:, kb, cs],
                            start=(kb == 0),
                            stop=(kb == NDB - 1),
                        )
                    qs = stage.tile([128, 512], BF16, tag="qs")
                    nc.vector.tensor_copy(qs, pq)
                    t1 = rtmp.tile([128, 512], BF16, tag="t1q")
                    t2 = rtmp.tile([128, 512], BF16, tag="t2q")
                    nc.vector.tensor_mul(t1, qs, cos2[:, cs])
                    for hh in range(2):
                        r = 64 * hh
                        nc.vector.tensor_mul(
                            t2[r:r + 32, :], qs[r + 32:r + 64, :],
                            sin2[r + 32:r + 64, cs],
                        )
                        nc.vector.tensor_mul(
                            t2[r + 32:r + 64, :], qs[r:r + 32, :],
                            sin2[r:r + 32, cs],
                        )
                    for hh in range(2):
                        h = pack * 2 + hh
                        r = 64 * hh
                        qcols = slice(h * N + ch * 512, h * N + (ch + 1) * 512)
                        if USE_FP8:
                            nc.vector.tensor_add(
                                q8[:, 0, qcols], t1[r:r + 32, :], t2[r:r + 32, :]
                            )
                            nc.vector.tensor_add(
                                q8[:, 1, qcols],
                                t1[r + 32:r + 64, :],
                                t2[r + 32:r + 64, :],
                            )
                        else:
                            nc.vector.tensor_add(
                                qt_sb[0:64, qcols],
                                t1[r:r + 64, :],
                                t2[r:r + 64, :],
                            )

            q_proj_ch(0)
            q_proj_ch(1)

            # ---- attention + interleaved output projection ----
            # PSUM: psS 2x[128,1024] (4 banks) + psO 3x[65,512] (3) + pq (1).
            # Out-projection reuses the psS buffers (same tag) in the gaps
            # between attention rounds.
            with (
                tc.tile_pool(name="ppool", bufs=3) as ppool,
                tc.tile_pool(name="rcp", bufs=2) as rcp,
                tc.tile_pool(name="bcp", bufs=2) as bcp,
                tc.tile_pool(name="psS", bufs=2, space="PSUM") as psS,
                tc.tile_pool(name="psO", bufs=3, space="PSUM") as psO,
                tc.tile_pool(name="ostage", bufs=2) as ostage,
            ):
                def attend(qh, h):
                    base = h * N + qh * 1024
                    po = [psO.tile([65, 512], F32, tag="o", name=f"po{qh}_{h}_{i}")
                          for i in range(2)]
                    for kb in range(NKB):
                        ps = psS.tile([128, 1024], F32, tag="s")
                        for half in range(2):
                            qcols = slice(base + half * 512, base + (half + 1) * 512)
                            if USE_FP8:
                                nc.tensor.matmul(
                                    ps[:, half * 512:(half + 1) * 512],
                                    k8[:, :, kb * 128:(kb + 1) * 128],
                                    q8[:, :, qcols],
                                    start=True,
                                    stop=True,
                                    perf_mode=DROW,
                                )
                            else:
                                nc.tensor.matmul(
                                    ps[:, half * 512:(half + 1) * 512],
                                    kt_sb[:, kb * 128:(kb + 1) * 128],
                                    qt_sb[:, qcols],
                                    start=True,
                                    stop=True,
                                )
                        p_t = ppool.tile([128, 1024], BF16, tag="p")
                        nc.scalar.activation(p_t, ps, EXP, bias=0.0, scale=SCALE)
                        for half in range(2):
                            nc.tensor.matmul(
                                po[half],
                                vaug_sb[:, kb, :],
                                p_t[:, half * 512:(half + 1) * 512],
                                start=(kb == 0),
                                stop=(kb == NKB - 1),
                                skip_group_check=True,
                            )
                    pair, row0 = h // 2, 64 * (h % 2)
                    for half in range(2):
                        rc = rcp.tile([1, 512], F32, tag="rc")
                        if os.environ.get("KERNEL_SLOW_RECIP") == "1":
                            nc.vector.reciprocal(rc, po[half][64:65, :])
                        else:
                            nc.vector.reciprocal_approx_fast(rc, po[half][64:65, :])
                        bc = bcp.tile([64, 512], F32, tag="bc")
                        nc.gpsimd.partition_broadcast(bc, rc)
                        ocols = slice(qh * 1024 + half * 512,
                                      qh * 1024 + (half + 1) * 512)
                        nc.vector.tensor_mul(
                            ot_sb[pair][row0:row0 + 64, ocols],
                            po[half][0:64, :],
                            bc,
                        )

                def out_proj(qh):
                    for qb in range(qh * 8, qh * 8 + 8):
                        pt = psS.tile([128, 1024], F32, tag="s")
                        for nch in range(2):
                            for pair in range(2):
                                nc.tensor.matmul(
                                    pt[:, nch * 512:(nch + 1) * 512],
                                    ot_sb[pair][:, qb * 128:(qb + 1) * 128],
                                    wo_sb[:, pair, nch * 512:(nch + 1) * 512],
                                    start=(pair == 0),
                                    stop=(pair == 1),
                                )
                        st = ostage.tile([128, 1024], F16, tag="st")
                        nc.vector.tensor_copy(st, pt)
                        for nch in range(2):
                            nc.sync.dma_start(
                                out[qb * 128:(qb + 1) * 128,
                                    nch * 512:(nch + 1) * 512],
                                st[:, nch * 512:(nch + 1) * 512],
                            )

                for h in range(HPC):
                    attend(0, h)
                q_proj_ch(2)
                q_proj_ch(3)
                out_proj(0)
                for h in range(HPC):
                    attend(1, h)
                out_proj(1)

    es.close()


def _rope_tables():
    inv_freq = 1.0 / (10000.0 ** (np.arange(0, D, 2, dtype=np.float64) / D))
    freqs = np.outer(np.arange(N, dtype=np.float64), inv_freq)  # [N, 32]
    cos_h = np.cos(freqs).astype(np.float32).T                  # [32, N]
    sin_h = np.sin(freqs).astype(np.float32).T                  # [32, N]
    cost = np.concatenate([cos_h, cos_h], 0)                    # [64, N]
    # sign layout [+sin; -sin]: the crossing multiplies index this table at
    # the SOURCE partitions (walrus requires equal base partitions for
    # SBUF+SBUF tensor_tensor inputs), so row r holds the sign of the row it
    # multiplies INTO the other half.
    sincat = np.concatenate([sin_h, -sin_h], 0)                 # [64, N]
    return np.ascontiguousarray(cost), np.ascontiguousarray(sincat)


@functools.lru_cache(maxsize=1)
def _program():
    nc = bacc.Bacc(
        "TRN2", target_bir_lowering=False, debug=False, enable_asserts=False
    )
    io = {
        "xt": nc.dram_tensor("xt", [NDB, 128, N], BF16, kind="ExternalInput").ap(),
        "wq": nc.dram_tensor("wq", [NDB, 128, 256], BF16, kind="ExternalInput").ap(),
        "wkv": nc.dram_tensor("wkv", [NDB, 128, 128], BF16, kind="ExternalInput").ap(),
        "wo": nc.dram_tensor("wo", [2, 128, DIM], BF16, kind="ExternalInput").ap(),
        "cost": nc.dram_tensor("cost", [64, N], BF16, kind="ExternalInput").ap(),
        "sincat": nc.dram_tensor("sincat", [64, N], BF16, kind="ExternalInput").ap(),
        "out": nc.dram_tensor("out", [N, DIM], F16, kind="ExternalOutput").ap(),
    }
    with tile.TileContext(nc) as tc:
        _build(nc, tc, io)
    nc.compile()
    return nc


def make_in_maps(x, Wq, Wkv, Wo):
    import ml_dtypes

    bf16 = ml_dtypes.bfloat16
    cost, sincat = _rope_tables()
    in_maps = []
    for c in range(8):
        b, j = c // 4, c % 4
        xt = np.ascontiguousarray(x[b].T).reshape(NDB, 128, N)
        wq_c = np.ascontiguousarray(Wq[:, 256 * j:256 * (j + 1)]).reshape(
            NDB, 128, 256
        )
        wkv_c = np.ascontiguousarray(
            np.concatenate(
                [Wkv[:, 64 * j:64 * (j + 1)],
                 Wkv[:, 256 + 64 * j:256 + 64 * (j + 1)]],
                axis=1,
            )
        ).reshape(NDB, 128, 128)
        wo_c = np.ascontiguousarray(Wo[256 * j:256 * (j + 1), :]).reshape(
            2, 128, DIM
        )
        in_maps.append(
            {
                "xt": xt.astype(bf16),
                "wq": wq_c.astype(bf16),
                "wkv": wkv_c.astype(bf16),
                "wo": wo_c.astype(bf16),
                "cost": cost.astype(bf16),
                "sincat": sincat.astype(bf16),
            }
        )
    return in_maps


def _install_ntff_hook():
    """Register the axon NTFF profiling hook that this image's antenv lacks."""
    import types

    if "antenv.axon_hooks" in sys.modules:
        return
    try:
        sys.path.append("/root/.axon_site")
        from trn_agent_boot.trn_boot import _ntff_profile_via_ctypes

        hook = _ntff_profile_via_ctypes("/opt/axon/libaxon_pjrt.so")
    except Exception:
        hook = None
    finally:
        try:
            sys.path.remove("/root/.axon_site")
        except ValueError:
            pass
    mod = types.ModuleType("antenv.axon_hooks")
    mod.get_axon_ntff_profile_hook = lambda: hook
    mod.set_axon_ntff_profile_hook = lambda h: None
    sys.modules["antenv.axon_hooks"] = mod
    # artifact upload needs bucket credentials this container lacks
    import concourse.bass_utils as bu

    bu.upload_artifacts = lambda tmpdir: "local://" + str(tmpdir)


def kernel(x, Wq, Wkv, Wo, bo):
    from concourse.bass_utils import run_bass_kernel_spmd

    _install_ntff_hook()
    nc = _program()
    in_maps = make_in_maps(x, Wq, Wkv, Wo)
    trace = bool(os.environ.get("KERNEL_TRACE"))
    res = run_bass_kernel_spmd(
        nc, in_maps, list(range(8)), trace=trace
    )
    LAST_RESULTS["res"] = res
    full = np.zeros((B, N, DIM), np.float32)
    for c in range(8):
        full[c // 4] += res.results[c]["out"].astype(np.float32)
    full += bo.astype(np.float32)
    return full


# revision 20
# speedup vs baseline: 1.1227x; 1.1010x over previous
"""GQA attention (16 Q heads / 4 KV heads, RoPE, n=2048, d=64) on 8 trn2 cores.

Sharding: core c = (batch b=c//4, kv-group j=c%4). Each core owns 4 query
heads sharing one KV head, computes its partial output projection
(O_heads @ Wo_rows), and the host sums the 4 partials per batch.

Pipeline per core (all matmul inputs bf16/fp8, fp32 PSUM accumulation):
  x DMA'd ch-major so the KV projection starts ~7us in.
  RoPE on DVE in bf16 2x mode (PSUM staged to SBUF via ACT copies).
  S^T = K^T Q as fp8-e4m3 DoubleRow matmuls (d=64 folded to 32 partitions
    x 2 k-slots) -- half the PE time of bf16; P = exp on ACT; O^T+denom
    from bf16 matmul with V_aug (ones column -> denominator).
  Output projection interleaved per 512-query chunk, fp16 output.
"""

import os
import sys
import functools

import numpy as np

sys.path.insert(0, "/opt/trn_rl_repo")

import concourse.bass as bass  # noqa: E402
import concourse.bacc as bacc  # noqa: E402
import concourse.tile as tile  # noqa: E402
import concourse.mybir as mybir  # noqa: E402
from concourse.masks import make_identity  # noqa: E402

F32 = mybir.dt.float32
BF16 = mybir.dt.bfloat16
F16 = mybir.dt.float16
FP8 = mybir.dt.float8e4
EXP = mybir.ActivationFunctionType.Exp
DROW = mybir.MatmulPerfMode.DoubleRow

B, N, DIM = 2, 2048, 1024
HEADS, KVH, D = 16, 4, 64
HPC = HEADS // KVH          # q heads per core = 4
SCALE = D ** -0.5           # 1/8
QTOT = HPC * N              # 8192 concatenated query columns
NKB = N // 128              # 16 key blocks
NDB = DIM // 128            # 8 contraction blocks for projections

USE_FP8 = os.environ.get("KERNEL_FP8") == "1"

LAST_RESULTS = {}           # test.py introspection


def _build(nc, tc, io):
    from contextlib import ExitStack

    xt, wq, wkv, wo = io["xt"], io["wq"], io["wkv"], io["wo"]
    cost, sincat, out = io["cost"], io["sincat"], io["out"]

    es = ExitStack()
    consts = es.enter_context(tc.tile_pool(name="consts", bufs=1))
    acts = es.enter_context(tc.tile_pool(name="acts", bufs=1))

    wq_sb = consts.tile([128, NDB, 256], BF16, tag="wq")
    wkv_sb = consts.tile([128, NDB, 128], BF16, tag="wkv")
    wo_sb = consts.tile([128, 2, DIM], BF16, tag="wo")
    cos2 = consts.tile([128, N], BF16, tag="cos2")
    sin2 = consts.tile([128, N], BF16, tag="sin2")
    id64 = consts.tile([64, 64], BF16, tag="id")

    vt_sb = acts.tile([64, N], BF16, tag="vt")
    vaug_sb = acts.tile([128, NKB, 65], BF16, tag="vaug")
    ot_sb = [
        acts.tile([128, N], BF16, tag=f"ot{i}", name=f"ot{i}") for i in range(2)
    ]
    if USE_FP8:
        k8 = acts.tile([32, 2, N], FP8, tag="k8")
        q8 = acts.tile([32, 2, QTOT], FP8, tag="q8")
    else:
        kt_sb = acts.tile([128, N], BF16, tag="kt")
        qt_sb = acts.tile([128, QTOT], BF16, tag="qt")

    with tc.tile_pool(name="xtp", bufs=1) as xt_pool:
        xt_sb = xt_pool.tile([128, NDB, N], BF16, tag="xt")

        # DMA issue order: KV weights + ch0 tables + ch0 x first.
        for g in range(4):
            nc.sync.dma_start(
                wkv_sb[:, 2 * g:2 * g + 2, :],
                wkv[2 * g:2 * g + 2].transpose([1, 0, 2]),
            )
        nc.sync.dma_start(cos2[0:64, 0:1024], cost[:, 0:1024])
        nc.sync.dma_start(sin2[0:64, 0:1024], sincat[:, 0:1024])
        for kb in range(NDB):
            nc.sync.dma_start(xt_sb[:, kb, 0:512], xt[kb, :, 0:512])
        for kb in range(NDB):
            nc.sync.dma_start(wq_sb[:, kb, :], wq[kb])
        nc.sync.dma_start(cos2[0:64, 1024:N], cost[:, 1024:N])
        nc.sync.dma_start(sin2[0:64, 1024:N], sincat[:, 1024:N])
        nc.sync.dma_start(cos2[64:128, 0:1024], cost[:, 0:1024])
        nc.sync.dma_start(sin2[64:128, 0:1024], sincat[:, 0:1024])
        nc.sync.dma_start(cos2[64:128, 1024:N], cost[:, 1024:N])
        nc.sync.dma_start(sin2[64:128, 1024:N], sincat[:, 1024:N])
        for ch in range(1, 4):
            for kb in range(NDB):
                nc.sync.dma_start(
                    xt_sb[:, kb, ch * 512:(ch + 1) * 512],
                    xt[kb, :, ch * 512:(ch + 1) * 512],
                )
        for pair in range(2):
            for half in range(2):
                nc.sync.dma_start(
                    wo_sb[:, pair, half * 512:(half + 1) * 512],
                    wo[pair][:, half * 512:(half + 1) * 512],
                )
        make_identity(nc, id64)
        if not USE_FP8:
            nc.gpsimd.memset(kt_sb[64:128, :], 0.0)
            nc.gpsimd.memset(qt_sb[64:128, :], 0.0)

        with (
            tc.tile_pool(name="stage", bufs=3) as stage,
            tc.tile_pool(name="rtmp", bufs=3) as rtmp,
            tc.tile_pool(name="pq", bufs=1, space="PSUM") as pqp,
        ):
            with (
                tc.tile_pool(name="pkv", bufs=2, space="PSUM") as pp,
                tc.tile_pool(name="ptr", bufs=2, space="PSUM") as ptp,
            ):
                # ---- KV projection + K rope + V staging ----
                for ch in range(4):
                    cs = slice(ch * 512, (ch + 1) * 512)
                    pkv = pp.tile([128, 512], F32, tag="pj")
                    for kb in range(NDB):
                        nc.tensor.matmul(
                            pkv,
                            wkv_sb[:, kb, :],
                            xt_sb[:, kb, cs],
                            start=(kb == 0),
                            stop=(kb == NDB - 1),
                        )
                    ks = stage.tile([64, 512], BF16, tag="ks")
                    nc.vector.tensor_copy(ks, pkv[0:64, :])
                    nc.vector.tensor_copy(vt_sb[:, cs], pkv[64:128, :])
                    t1 = rtmp.tile([64, 512], BF16, tag="t1")
                    t2 = rtmp.tile([64, 512], BF16, tag="t2")
                    nc.vector.tensor_mul(t1, ks, cos2[0:64, cs])
                    nc.vector.tensor_mul(t2[0:32, :], ks[32:64, :], sin2[32:64, cs])
                    nc.vector.tensor_mul(t2[32:64, :], ks[0:32, :], sin2[0:32, cs])
                    if USE_FP8:
                        nc.vector.tensor_add(k8[:, 0, cs], t1[0:32, :], t2[0:32, :])
                        nc.vector.tensor_add(k8[:, 1, cs], t1[32:64, :], t2[32:64, :])
                    else:
                        nc.vector.tensor_add(kt_sb[0:64, cs], t1, t2)

                # ---- V_aug: transpose vT -> [keys,64] blocks + ones column ----
                for t in range(NKB):
                    ptr = ptp.tile([128, 64], BF16, tag="pjt")
                    nc.tensor.transpose(
                        ptr, vt_sb[:, t * 128:(t + 1) * 128], id64
                    )
                    nc.vector.tensor_copy(vaug_sb[:, t, 0:64], ptr)
                    nc.vector.memset(vaug_sb[:, t, 64:65], 1.0)

            # ---- Q projection + rope (pipelined into the attention rounds) ----
            def q_proj_ch(ch):
                cs = slice(ch * 512, (ch + 1) * 512)
                for pack in range(2):
                    pq = pqp.tile([128, 512], F32, tag="pj")
                    for kb in range(NDB):
                        nc.tensor.matmul(
                            pq,
                            wq_sb[:, kb, pack * 128:(pack + 1) * 128],
                            xt_sb[:, kb, cs],
                            start=(kb == 0),
                            stop=(kb == NDB - 1),
                        )
                    qs = stage.tile([128, 512], BF16, tag="qs")
                    nc.vector.tensor_copy(qs, pq)
                    t1 = rtmp.tile([128, 512], BF16, tag="t1q")
                    t2 = rtmp.tile([128, 512], BF16, tag="t2q")
                    nc.vector.tensor_mul(t1, qs, cos2[:, cs])
                    for hh in range(2):
                        r = 64 * hh
                        nc.vector.tensor_mul(
                            t2[r:r + 32, :], qs[r + 32:r + 64, :],
                            sin2[r + 32:r + 64, cs],
                        )
                        nc.vector.tensor_mul(
                            t2[r + 32:r + 64, :], qs[r:r + 32, :],
                            sin2[r:r + 32, cs],
                        )
                    for hh in range(2):
                        h = pack * 2 + hh
                        r = 64 * hh
                        qcols = slice(h * N + ch * 512, h * N + (ch + 1) * 512)
                        if USE_FP8:
                            nc.vector.tensor_add(
                                q8[:, 0, qcols], t1[r:r + 32, :], t2[r:r + 32, :]
                            )
                            nc.vector.tensor_add(
                                q8[:, 1, qcols],
                                t1[r + 32:r + 64, :],
                                t2[r + 32:r + 64, :],
                            )
                        else:
                            nc.vector.tensor_add(
                                qt_sb[0:64, qcols],
                                t1[r:r + 64, :],
                                t2[r:r + 64, :],
                            )

            q_proj_ch(0)
            q_proj_ch(1)

            # ---- attention + interleaved output projection ----
            # PSUM: psS 2x[128,1024] (4 banks) + psO 3x[65,512] (3) + pq (1).
            # Out-projection reuses the psS buffers (same tag) in the gaps
            # between attention rounds.
            with (
                tc.tile_pool(name="ppool", bufs=3) as ppool,
                tc.tile_pool(name="unnp", bufs=10) as unnp,
                tc.tile_pool(name="den8p", bufs=2) as den8p,
                tc.tile_pool(name="rc8p", bufs=2) as rc8p,
                tc.tile_pool(name="bcp", bufs=2) as bcp,
                tc.tile_pool(name="psS", bufs=2, space="PSUM") as psS,
                tc.tile_pool(name="psO", bufs=3, space="PSUM") as psO,
                tc.tile_pool(name="ostage", bufs=2) as ostage,
            ):
                def attend(qh, h, den8, unns):
                    base = h * N + qh * 1024
                    po = [psO.tile([65, 512], F32, tag="o", name=f"po{qh}_{h}_{i}")
                          for i in range(2)]
                    for kb in range(NKB):
                        ps = psS.tile([128, 1024], F32, tag="s")
                        for half in range(2):
                            qcols = slice(base + half * 512, base + (half + 1) * 512)
                            if USE_FP8:
                                nc.tensor.matmul(
                                    ps[:, half * 512:(half + 1) * 512],
                                    k8[:, :, kb * 128:(kb + 1) * 128],
                                    q8[:, :, qcols],
                                    start=True,
                                    stop=True,
                                    perf_mode=DROW,
                                )
                            else:
                                nc.tensor.matmul(
                                    ps[:, half * 512:(half + 1) * 512],
                                    kt_sb[:, kb * 128:(kb + 1) * 128],
                                    qt_sb[:, qcols],
                                    start=True,
                                    stop=True,
                                )
                        p_t = ppool.tile([128, 1024], BF16, tag="p")
                        nc.scalar.activation(p_t, ps, EXP, bias=0.0, scale=SCALE)
                        for half in range(2):
                            nc.tensor.matmul(
                                po[half],
                                vaug_sb[:, kb, :],
                                p_t[:, half * 512:(half + 1) * 512],
                                start=(kb == 0),
                                stop=(kb == NKB - 1),
                                skip_group_check=True,
                            )
                    # stash unnormalized O + denominator rows; PSUM frees now,
                    # the divide happens via batched reciprocals per qh.
                    # SBUF APs may only start at partition 0/32/64/96, so the
                    # 8 denominator rows go to rows {0,32,64,96} of 2 tiles.
                    for half in range(2):
                        r = h * 2 + half
                        unn = unnp.tile([64, 512], BF16, tag="unn",
                                        name=f"unn{qh}_{r}")
                        nc.vector.tensor_copy(unn, po[half][0:64, :])
                        row = (r % 4) * 32
                        nc.vector.tensor_copy(
                            den8[r // 4][row:row + 1, :], po[half][64:65, :]
                        )
                        unns.append(unn)

                def normalize(qh, den8, unns):
                    rec = []
                    for t in range(2):
                        rc = rc8p.tile([97, 512], F32, tag="rc8")
                        nc.vector.reciprocal(rc, den8[t])
                        rec.append(rc)
                    for h in range(HPC):
                        pair, row0 = h // 2, 64 * (h % 2)
                        for half in range(2):
                            r = h * 2 + half
                            row = (r % 4) * 32
                            # hw partition_broadcast reads physical partition
                            # 0, so bounce the row to a base-0 tile first
                            rr = rc8p.tile([1, 512], F32, tag="rr")
                            nc.vector.tensor_copy(rr, rec[r // 4][row:row + 1, :])
                            bc = bcp.tile([64, 512], F32, tag="bc")
                            nc.gpsimd.partition_broadcast(bc, rr)
                            ocols = slice(qh * 1024 + half * 512,
                                          qh * 1024 + (half + 1) * 512)
                            nc.vector.tensor_mul(
                                ot_sb[pair][row0:row0 + 64, ocols],
                                unns[r],
                                bc,
                            )

                def out_proj(qh):
                    for qb in range(qh * 8, qh * 8 + 8):
                        pt = psS.tile([128, 1024], F32, tag="s")
                        for nch in range(2):
                            for pair in range(2):
                                nc.tensor.matmul(
                                    pt[:, nch * 512:(nch + 1) * 512],
                                    ot_sb[pair][:, qb * 128:(qb + 1) * 128],
                                    wo_sb[:, pair, nch * 512:(nch + 1) * 512],
                                    start=(pair == 0),
                                    stop=(pair == 1),
                                )
                        st = ostage.tile([128, 1024], F16, tag="st")
                        nc.vector.tensor_copy(st, pt)
                        for nch in range(2):
                            nc.sync.dma_start(
                                out[qb * 128:(qb + 1) * 128,
                                    nch * 512:(nch + 1) * 512],
                                st[:, nch * 512:(nch + 1) * 512],
                            )

                den8_0 = [den8p.tile([97, 512], F32, tag="d8", name=f"den8_0{t}")
                          for t in range(2)]
                for t in range(2):
                    nc.vector.memset(den8_0[t], 1.0)
                unns_0 = []
                for h in range(HPC):
                    attend(0, h, den8_0, unns_0)
                q_proj_ch(2)
                q_proj_ch(3)
                normalize(0, den8_0, unns_0)
                out_proj(0)
                den8_1 = [den8p.tile([97, 512], F32, tag="d8", name=f"den8_1{t}")
                          for t in range(2)]
                for t in range(2):
                    nc.vector.memset(den8_1[t], 1.0)
                unns_1 = []
                for h in range(HPC):
                    attend(1, h, den8_1, unns_1)
                normalize(1, den8_1, unns_1)
                out_proj(1)

    es.close()


def _rope_tables():
    inv_freq = 1.0 / (10000.0 ** (np.arange(0, D, 2, dtype=np.float64) / D))
    freqs = np.outer(np.arange(N, dtype=np.float64), inv_freq)  # [N, 32]
    cos_h = np.cos(freqs).astype(np.float32).T                  # [32, N]
    sin_h = np.sin(freqs).astype(np.float32).T                  # [32, N]
    cost = np.concatenate([cos_h, cos_h], 0)                    # [64, N]
    # sign layout [+sin; -sin]: the crossing multiplies index this table at
    # the SOURCE partitions (walrus requires equal base partitions for
    # SBUF+SBUF tensor_tensor inputs), so row r holds the sign of the row it
    # multiplies INTO the other half.
    sincat = np.concatenate([sin_h, -sin_h], 0)                 # [64, N]
    return np.ascontiguousarray(cost), np.ascontiguousarray(sincat)


@functools.lru_cache(maxsize=1)
def _program():
    nc = bacc.Bacc(
        "TRN2", target_bir_lowering=False, debug=False, enable_asserts=False
    )
    io = {
        "xt": nc.dram_tensor("xt", [NDB, 128, N], BF16, kind="ExternalInput").ap(),
        "wq": nc.dram_tensor("wq", [NDB, 128, 256], BF16, kind="ExternalInput").ap(),
        "wkv": nc.dram_tensor("wkv", [NDB, 128, 128], BF16, kind="ExternalInput").ap(),
        "wo": nc.dram_tensor("wo", [2, 128, DIM], BF16, kind="ExternalInput").ap(),
        "cost": nc.dram_tensor("cost", [64, N], BF16, kind="ExternalInput").ap(),
        "sincat": nc.dram_tensor("sincat", [64, N], BF16, kind="ExternalInput").ap(),
        "out": nc.dram_tensor("out", [N, DIM], F16, kind="ExternalOutput").ap(),
    }
    with tile.TileContext(nc) as tc:
        _build(nc, tc, io)
    nc.compile()
    return nc


def make_in_maps(x, Wq, Wkv, Wo):
    import ml_dtypes

    bf16 = ml_dtypes.bfloat16
    cost, sincat = _rope_tables()
    in_maps = []
    for c in range(8):
        b, j = c // 4, c % 4
        xt = np.ascontiguousarray(x[b].T).reshape(NDB, 128, N)
        wq_c = np.ascontiguousarray(Wq[:, 256 * j:256 * (j + 1)]).reshape(
            NDB, 128, 256
        )
        wkv_c = np.ascontiguousarray(
            np.concatenate(
                [Wkv[:, 64 * j:64 * (j + 1)],
                 Wkv[:, 256 + 64 * j:256 + 64 * (j + 1)]],
                axis=1,
            )
        ).reshape(NDB, 128, 128)
        wo_c = np.ascontiguousarray(Wo[256 * j:256 * (j + 1), :]).reshape(
            2, 128, DIM
        )
        in_maps.append(
            {
                "xt": xt.astype(bf16),
                "wq": wq_c.astype(bf16),
                "wkv": wkv_c.astype(bf16),
                "wo": wo_c.astype(bf16),
                "cost": cost.astype(bf16),
                "sincat": sincat.astype(bf16),
            }
        )
    return in_maps


def _install_ntff_hook():
    """Register the axon NTFF profiling hook that this image's antenv lacks."""
    import types

    if "antenv.axon_hooks" in sys.modules:
        return
    try:
        sys.path.append("/root/.axon_site")
        from trn_agent_boot.trn_boot import _ntff_profile_via_ctypes

        hook = _ntff_profile_via_ctypes("/opt/axon/libaxon_pjrt.so")
    except Exception:
        hook = None
    finally:
        try:
            sys.path.remove("/root/.axon_site")
        except ValueError:
            pass
    mod = types.ModuleType("antenv.axon_hooks")
    mod.get_axon_ntff_profile_hook = lambda: hook
    mod.set_axon_ntff_profile_hook = lambda h: None
    sys.modules["antenv.axon_hooks"] = mod
    # artifact upload needs bucket credentials this container lacks
    import concourse.bass_utils as bu

    bu.upload_artifacts = lambda tmpdir: "local://" + str(tmpdir)


def kernel(x, Wq, Wkv, Wo, bo):
    from concourse.bass_utils import run_bass_kernel_spmd

    _install_ntff_hook()
    nc = _program()
    in_maps = make_in_maps(x, Wq, Wkv, Wo)
    trace = bool(os.environ.get("KERNEL_TRACE"))
    res = run_bass_kernel_spmd(
        nc, in_maps, list(range(8)), trace=trace
    )
    LAST_RESULTS["res"] = res
    full = np.zeros((B, N, DIM), np.float32)
    for c in range(8):
        full[c // 4] += res.results[c]["out"].astype(np.float32)
    full += bo.astype(np.float32)
    return full
